# revision 19
# baseline (speedup 1.0000x reference)
"""CRF loss (nn_CRF) Trainium2 kernel.

B=128, S=2048, T=64. loss = -(mean_b(score_b - logZ_b)).

Strategy (sequence-parallel chunked forward algorithm):
  The forward logsumexp recurrence contracts initial-condition differences by
  ~7x per step (dense random transitions), so the 2048-step scan is split into
  64 independent chains (8 per core x 8 cores), each covering K=32 global
  steps. Chains start from an arbitrary state; log-space maps are
  additive-homogeneous, so each chain's output equals the true alpha up to one
  scalar per batch, recovered on the host by telescoping lse differences at
  the chain boundaries (the contraction makes the rank-1 level-transfer
  identity accurate to ~1e-4 overall).

  On device the scan runs in exp space: g <- exp(em) * (W'^T @ g) with
  W' = exp(transitions - C0) (the per-step rescale exp(-C0) keeps magnitudes
  bounded; it is folded into the stationary weights). Chain q's initial state
  is one DVE tensor_scalar: h_0 = exp(em row 32q) * colsum(W') = the true
  first-step image of the all-ones vector, so only NSTEP=31 matmul steps
  remain. Chain 0 anchors the absolute level: its slab row 0 is solved so h_0
  lands exactly on an exp-space representative of alpha_0 = start + em[:,0]
  (shifted by a host scalar mshift to stay inside fp8 range).

  Per core the 8 chains run as 4 lockstep groups of 2 chains fused in the
  matmul free dim: each group-step is ONE 128x128x128 bf16 matmul (block-diag
  W', two 64-batch groups in partitions, 2 chains side by side in the free
  dim) plus ONE DVE tensor-tensor multiply (PSUM fp32 x Em fp8 -> SBUF bf16).
  The 4 groups rotate so the PE->DVE->PE round-trip (~750ns) hides under the
  4-TT issue pitch; the schedule is DVE-issue-bound at ~(128+58)/0.96 ns per
  TT with zero bubbles.

  Emissions are uploaded as exp(em) in fp8-e4m3 (clamped to 224: the device
  decodes e4m3 as IEEE, inf at >=256), laid out STEP-MAJOR so each DMA
  descriptor is a multi-KB contiguous run per partition and data arrives in
  consumption order on the two HWDGE queues (sync/scalar).

  The program is built WITHOUT TileContext: explicit semaphores (~11) and
  embedded instruction waits keep the per-engine sync instruction count
  minimal; output-DMA completions go to a semaphore nothing waits on, so the
  NEFF epilogue's sem-file zeroing cannot race anything that matters.

  The gold-path score (gathers over tags) is O(B*S) trivial index work and is
  computed on the host in fp64, as is the final stitching.
"""

import numpy as np
from contextlib import ExitStack

B, S, T = 128, 2048, 64
NCORE = 8
K = 32             # global steps covered per chain
NSTEP = K - 1      # device matmul steps (step 0 is the tensor_scalar init)
NCH = 8            # chains per core
GROUPS = [2, 2, 2, 2]  # chains per lockstep group (4 groups hide the latency)
NGRP = len(GROUPS)
GOFF = [64 * sum(GROUPS[:g]) for g in range(NGRP)]
GWID = [64 * n for n in GROUPS]
SW = NCH * 64      # slab free width per row (512)
NCHAIN = NCORE * NCH
C0 = np.float32(5.45)

# row boundaries of the em-slab DMA chunks (slab row r = global em row
# 32q + r; row 0 feeds the init, row r>=1 feeds device step r-1). First
# chunks small so compute can start early; round-robin over the two HWDGE
# queues (even chunks -> scalar, odd -> sync behind wexp).
_CHUNK_ROWS = [0, 2, 6, 12, 18, 25, K]

_prog_cache = {}
_last_results = None


# ----------------------------------------------------------------------------
# device program (built once, cached)
# ----------------------------------------------------------------------------

def _embed_wait(mybir, inst, sem, val):
    """Attach a >= semaphore wait directly to an instruction (1 wait max)."""
    si = inst.ins.sync_info
    upd = list(si.on_update) if (si is not None and si.on_update) else []
    wts = list(si.on_wait) if (si is not None and si.on_wait) else []
    assert not wts
    wts.append(mybir.SyncWait(sync_type="semaphore", id=sem.num, ant_name="w",
                              wait_mode="sem-ge-imm", wait_value=val,
                              wait_reg=None))
    inst.ins.sync_info = mybir.SyncInfo(on_wait=wts, on_update=upd)
    return inst


def _build_program():
    import concourse.bass as bass
    from concourse import mybir

    nc = bass.Bass("TRN2", target_bir_lowering=False, debug=False,
                   num_devices=NCORE)
    em_slab = nc.dram_tensor("em_slab", [128, K * SW], mybir.dt.float8e4,
                             kind="ExternalInput").ap()
    wexp = nc.dram_tensor("wexp", [128, 128], mybir.dt.bfloat16,
                          kind="ExternalInput").ap()
    csum_d = nc.dram_tensor("csum", [128, 1], mybir.dt.float32,
                            kind="ExternalInput").ap()
    out = nc.dram_tensor("out", [128, SW], mybir.dt.bfloat16,
                         kind="ExternalOutput").ap()

    FP32 = mybir.dt.float32
    BF16 = mybir.dt.bfloat16
    FP8 = mybir.dt.float8e4
    MULT = mybir.AluOpType.mult

    wt = nc.alloc_sbuf_tensor("wt", [128, 128], BF16).ap()
    csum = nc.alloc_sbuf_tensor("csum_sb", [128, 1], FP32).ap()
    em = nc.alloc_sbuf_tensor("em", [128, K * SW], FP8).ap()
    st0 = nc.alloc_sbuf_tensor("st0", [128, SW], BF16).ap()
    st = {g: [st0[:, GOFF[g]: GOFF[g] + GWID[g]],
              nc.alloc_sbuf_tensor(f"st{g}1", [128, GWID[g]], BF16).ap()]
          for g in range(NGRP)}
    ps = {g: [nc.alloc_psum_tensor(f"ps{g}{i}", [128, GWID[g]], FP32).ap()
              for i in range(2)] for g in range(NGRP)}

    with ExitStack() as ctx:
        tt_sem = [ctx.enter_context(nc.semaphore(f"tt{g}"))
                  for g in range(NGRP)]
        mm_sem = [ctx.enter_context(nc.semaphore(f"mm{g}"))
                  for g in range(NGRP)]
        qsy = ctx.enter_context(nc.semaphore("qsy"))
        qsc = ctx.enter_context(nc.semaphore("qsc"))
        # a-DMA completions go to `aq`, which nothing ever waits on, so a
        # completion racing the NEFF-epilogue sem zeroing is harmless
        aq = ctx.enter_context(nc.semaphore("aq"))

        # chunk j queue + completion count: even chunks on scalar behind
        # wexp (wexp=16, chunk0=32, chunk2=48, chunk4=64), odd on sync
        # behind csum (csum=16, chunk1=32, chunk3=48, chunk5=64)
        nchunk = len(_CHUNK_ROWS) - 1
        chunk_q = [(qsc, 16 * (j // 2 + 2)) if j % 2 == 0
                   else (qsy, 16 * (j // 2 + 2)) for j in range(nchunk)]
        chunk_of_row = {}
        for j in range(nchunk):
            chunk_of_row[_CHUNK_ROWS[j]] = j

        with nc.Block("crf", no_gpsimd_drain=True) as block:

            def f_sync(eng):
                eng.dma_start(csum, csum_d).then_inc(qsy, 16)
                for j in range(1, nchunk, 2):
                    sl = slice(_CHUNK_ROWS[j] * SW, _CHUNK_ROWS[j + 1] * SW)
                    eng.dma_start(em[:, sl], em_slab[:, sl]).then_inc(qsy, 16)
                for g in (1, 3):
                    eng.wait_ge(tt_sem[g], NSTEP + 1)
                    eng.dma_start(out[:, GOFF[g]: GOFF[g] + GWID[g]],
                                  st[g][NSTEP % 2]).then_inc(aq, 16)

            def f_scalar(eng):
                eng.dma_start(wt, wexp).then_inc(qsc, 16)
                for j in range(0, nchunk, 2):
                    sl = slice(_CHUNK_ROWS[j] * SW, _CHUNK_ROWS[j + 1] * SW)
                    eng.dma_start(em[:, sl], em_slab[:, sl]).then_inc(qsc, 16)
                for g in (0, 2):
                    eng.wait_ge(tt_sem[g], NSTEP + 1)
                    eng.dma_start(out[:, GOFF[g]: GOFF[g] + GWID[g]],
                                  st[g][NSTEP % 2]).then_inc(aq, 16)

            def f_tensor(eng):
                eng.wait_ge(qsc, 16)          # wexp loaded
                for k in range(NSTEP):
                    cur = k % 2
                    for g in range(NGRP):
                        mm = eng.matmul(ps[g][cur], wt, st[g][cur],
                                        start=True, stop=True)
                        # st[g][cur] written by tensor_scalar (k=0) / TT
                        _embed_wait(mybir, mm, tt_sem[g], k + 1)
                        mm.then_inc(mm_sem[g], 1)

            def f_vector(eng):
                # init: h_0 = (slab row 0) * colsum(W'), one per group
                eng.wait_ge(qsy, 16)          # csum loaded
                eng.wait_ge(*chunk_q[0])
                for g in range(NGRP):
                    eng.tensor_scalar(
                        st[g][0], em[:, GOFF[g]: GOFF[g] + GWID[g]],
                        csum, None, MULT).then_inc(tt_sem[g], 1)
                for k in range(NSTEP):
                    cur = k % 2
                    r = k + 1                 # slab row consumed by step k
                    if r in chunk_of_row:
                        eng.wait_ge(*chunk_q[chunk_of_row[r]])
                    for g in range(NGRP):
                        emk = em[:, SW * r + GOFF[g]:
                                 SW * r + GOFF[g] + GWID[g]]
                        tt = eng.tensor_tensor(st[g][(k + 1) % 2], ps[g][cur],
                                               emk, MULT)
                        _embed_wait(mybir, tt, mm_sem[g], k + 1)
                        tt.then_inc(tt_sem[g], 1)

            # output-DMA completion and semaphore zeroing are covered by the
            # NEFF's end-of-program drains + full sem-file zeroing epilogue
            block.sync(f_sync)
            block.scalar(f_scalar)
            block.tensor(f_tensor)
            block.vector(f_vector)

    return nc


def _get_program():
    if "nc" not in _prog_cache:
        _prog_cache["nc"] = _build_program()
    return _prog_cache["nc"]


# ----------------------------------------------------------------------------
# host-side helpers
# ----------------------------------------------------------------------------

def _bf16_f32(x):
    import ml_dtypes
    return np.asarray(x, np.float32).astype(ml_dtypes.bfloat16).astype(np.float32)


def _dev5(arr):
    """arr[K, NCH, B, T] fp32 -> device layout [128, K*SW]:
    p = 64*(b//64) + j, free offset = SW*r + 64*ch + b%64."""
    a = arr.reshape(K, NCH, 2, 64, T)          # r, ch, bg, b64, j
    a = a.transpose(2, 4, 0, 1, 3)             # bg, j, r, ch, b64
    return np.ascontiguousarray(a).reshape(128, K * SW)


def _build_slabs(emissions, start_t, csum):
    """Per-core Em slabs exp(em) in device layout (slab row r of chain q =
    global em row 32q + r; cast to fp8 at upload). Chain 0's row 0 is solved
    so the init tensor_scalar lands exactly on an exp-space representative of
    alpha_0 = start + em[:, 0] shifted by mshift."""
    em32 = emissions.astype(np.float32)
    slabs = np.empty((NCORE, K, NCH, B, T), np.float32)
    for c in range(NCORE):
        for ch in range(NCH):
            t0 = K * (NCH * c + ch)
            slabs[c, :, ch] = np.exp(em32[:, t0: t0 + K].transpose(1, 0, 2))
    a0 = start_t[None, :].astype(np.float32) + em32[:, 0] - np.log(csum)[None, :]
    mshift = np.float32(a0.max() - 5.3)
    slabs[0, 0, 0] = np.exp(a0 - mshift)
    # device decodes fp8e4 as IEEE e4m3 (inf at >=256); clamp to stay finite
    np.minimum(slabs, np.float32(224.0), out=slabs)
    return np.stack([_dev5(slabs[c]) for c in range(NCORE)]), mshift


def _lse64(v):
    m = v.max(-1)
    return m + np.log(np.exp(v - m[..., None]).sum(-1))


def _host_score(emissions, tags, transitions, start_t, end_t, mask):
    em64 = emissions.astype(np.float64)
    W64 = transitions.astype(np.float64)
    maskf = mask.astype(np.float64)
    emit = np.take_along_axis(em64, tags[..., None].astype(np.int64),
                              axis=2)[..., 0]
    trans = W64[tags[:, 1:], tags[:, :-1]]
    score = (start_t.astype(np.float64)[tags[:, 0]] + emit[:, 0]
             + ((trans + emit[:, 1:]) * maskf[:, 1:]).sum(1))
    last_idx = maskf.sum(1).astype(np.int64) - 1
    last_tags = np.take_along_axis(tags, last_idx[:, None], axis=1)[:, 0]
    return score + end_t.astype(np.float64)[last_tags]


def _fallback_reference(emissions, tags, mask, transitions, start_t, end_t):
    """Exact host computation (only used if mask is not all ones)."""
    em = emissions.astype(np.float64)
    Wt = transitions.astype(np.float64)
    alpha = start_t.astype(np.float64)[None, :] + em[:, 0]
    for t in range(1, S):
        x = alpha[:, :, None] + Wt[None]
        m = x.max(1)
        na = m + np.log(np.exp(x - m[:, None, :]).sum(1)) + em[:, t]
        alpha = np.where(mask[:, t][:, None], na, alpha)
    logZ = _lse64(alpha + end_t.astype(np.float64)[None, :])
    score = _host_score(emissions, tags, transitions, start_t, end_t, mask)
    return np.float32(-(score - logZ).mean())


# ----------------------------------------------------------------------------
# entry point
# ----------------------------------------------------------------------------

def kernel(emissions, tags, mask, transitions, start_transitions,
           end_transitions):
    global _last_results
    emissions = np.asarray(emissions, np.float32)
    tags = np.asarray(tags)
    mask = np.asarray(mask)
    transitions = np.asarray(transitions, np.float32)
    start_t = np.asarray(start_transitions, np.float32)
    end_t = np.asarray(end_transitions, np.float32)

    if not mask.all():
        return _fallback_reference(emissions, tags, mask, transitions,
                                   start_t, end_t)

    # --- host prep ---
    import ml_dtypes
    Wexp2 = np.zeros((128, 128), np.float32)
    Wexp2[:64, :64] = np.exp(transitions - C0)
    Wexp2[64:, 64:] = Wexp2[:64, :64]
    Wd = _bf16_f32(Wexp2)
    # csum as the device multiplies it (fp32 column sums of bf16 W')
    csum_dev = Wd[:64, :64].sum(0).astype(np.float32)
    slabs, mshift = _build_slabs(emissions, start_t, csum_dev)
    csum_col = np.concatenate([csum_dev, csum_dev]).reshape(128, 1)

    in_maps = [{"em_slab": slabs[c].astype(ml_dtypes.float8_e4m3fn),
                "wexp": Wd.astype(ml_dtypes.bfloat16),
                "csum": csum_col.astype(np.float32)}
               for c in range(NCORE)]

    # --- device run ---
    import os
    from concourse.bass_utils import run_bass_kernel_spmd
    nc = _get_program()
    res = run_bass_kernel_spmd(
        nc, in_maps, list(range(NCORE)),
        trace=bool(os.environ.get("CRF_TRACE")),
    )
    _last_results = res

    # --- unpack: chain q = NCH*core + ch at out cols [64*ch, 64*ch+64) ---
    a = np.zeros((NCHAIN, B, T), np.float64)
    for core in range(NCORE):
        o = np.asarray(res.results[core]["out"], np.float32)
        for ch in range(NCH):
            q = NCH * core + ch
            at_ = o[:, 64 * ch: 64 * ch + 64]
            for bg in range(2):
                a[q, 64 * bg: 64 * bg + 64] = at_[64 * bg: 64 * bg + 64].T

    # --- stitch (fp64); chain q's init is the first-step image of the
    #     all-ones vector, whose lse is log T ---
    with np.errstate(divide="ignore"):
        la = np.log(a)
    gam = np.zeros(B)
    La = la[0] + float(C0) * NSTEP + float(mshift)
    for q in range(1, NCHAIN):
        gam = gam + _lse64(La) - np.log(T)
        La = la[q] + float(C0) * K
    logZ = _lse64(La + end_t.astype(np.float64)[None, :]) + gam

    score = _host_score(emissions, tags, transitions, start_t, end_t, mask)
    return np.float32(-(score - logZ).mean())


# revision 20
# speedup vs baseline: 1.0554x; 1.0554x over previous
"""CRF loss (nn_CRF) Trainium2 kernel.

B=128, S=2048, T=64. loss = -(mean_b(score_b - logZ_b)).

Strategy (sequence-parallel chunked forward algorithm):
  The forward logsumexp recurrence contracts initial-condition differences by
  ~7x per step (dense random transitions), so the 2048-step scan is split into
  64 independent chains (8 per core x 8 cores), each covering K=32 global
  steps. Chains start from an arbitrary state; log-space maps are
  additive-homogeneous, so each chain's output equals the true alpha up to one
  scalar per batch, recovered on the host by telescoping lse differences at
  the chain boundaries (the contraction makes the rank-1 level-transfer
  identity accurate to ~1e-4 overall).

  On device the scan runs in exp space: g <- exp(em) * (W'^T @ g) with
  W' = exp(transitions - C0) (the per-step rescale exp(-C0) keeps magnitudes
  bounded; it is folded into the stationary weights). Chain q's initial state
  is one DVE tensor_scalar: h_0 = exp(em row 32q) * colsum(W') = the true
  first-step image of the all-ones vector, so only NSTEP=31 matmul steps
  remain. Chain 0 anchors the absolute level: its slab row 0 is solved so h_0
  lands exactly on an exp-space representative of alpha_0 = start + em[:,0]
  (shifted by a host scalar mshift to stay inside fp8 range).

  Per core the 8 chains run as 4 lockstep groups of 2 chains fused in the
  matmul free dim: each group-step is ONE 128x128x128 bf16 matmul (block-diag
  W', two 64-batch groups in partitions, 2 chains side by side in the free
  dim) plus ONE DVE tensor-tensor multiply (PSUM fp32 x Em fp8 -> SBUF bf16).
  The 4 groups rotate so the PE->DVE->PE round-trip (~750ns) hides under the
  4-TT issue pitch; the schedule is DVE-issue-bound at ~(128+58)/0.96 ns per
  TT with zero bubbles.

  Emissions are uploaded as exp(em) in fp8-e4m3 (clamped to 224: the device
  decodes e4m3 as IEEE, inf at >=256), laid out STEP-MAJOR so each DMA
  descriptor is a multi-KB contiguous run per partition and data arrives in
  consumption order on the two HWDGE queues (sync/scalar).

  The program is built WITHOUT TileContext: explicit semaphores (~11) and
  embedded instruction waits keep the per-engine sync instruction count
  minimal; output-DMA completions go to a semaphore nothing waits on, so the
  NEFF epilogue's sem-file zeroing cannot race anything that matters.

  The gold-path score (gathers over tags) is O(B*S) trivial index work and is
  computed on the host in fp64, as is the final stitching.
"""

import numpy as np
from contextlib import ExitStack

B, S, T = 128, 2048, 64
NCORE = 8
K = 32             # global steps covered per chain
NSTEP = K - 1      # device matmul steps (step 0 is the tensor_scalar init)
NCH = 8            # chains per core
GROUPS = [2, 2, 2, 2]  # chains per lockstep group (4 groups hide the latency)
NGRP = len(GROUPS)
GOFF = [64 * sum(GROUPS[:g]) for g in range(NGRP)]
GWID = [64 * n for n in GROUPS]
SW = NCH * 64      # slab free width per row (512)
NCHAIN = NCORE * NCH
C0 = np.float32(5.45)

# row boundaries of the em-slab DMA chunks (slab row r = global em row
# 32q + r; row 0 feeds the init, row r>=1 feeds device step r-1). First
# chunks small so compute can start early; round-robin over the two HWDGE
# queues (even chunks -> scalar, odd -> sync behind wexp).
_CHUNK_ROWS = [0, 2, 6, 12, 18, 25, K]

_prog_cache = {}
_last_results = None


# ----------------------------------------------------------------------------
# device program (built once, cached)
# ----------------------------------------------------------------------------

def _embed_wait(mybir, inst, sem, val):
    """Attach a >= semaphore wait directly to an instruction (1 wait max)."""
    si = inst.ins.sync_info
    upd = list(si.on_update) if (si is not None and si.on_update) else []
    wts = list(si.on_wait) if (si is not None and si.on_wait) else []
    assert not wts
    wts.append(mybir.SyncWait(sync_type="semaphore", id=sem.num, ant_name="w",
                              wait_mode="sem-ge-imm", wait_value=val,
                              wait_reg=None))
    inst.ins.sync_info = mybir.SyncInfo(on_wait=wts, on_update=upd)
    return inst


def _build_program():
    import concourse.bass as bass
    from concourse import mybir

    nc = bass.Bass("TRN2", target_bir_lowering=False, debug=False,
                   num_devices=NCORE)
    em_slab = nc.dram_tensor("em_slab", [128, K * SW], mybir.dt.float8e4,
                             kind="ExternalInput").ap()
    wexp = nc.dram_tensor("wexp", [128, 128], mybir.dt.bfloat16,
                          kind="ExternalInput").ap()
    out = nc.dram_tensor("out", [128, SW], mybir.dt.bfloat16,
                         kind="ExternalOutput").ap()

    FP32 = mybir.dt.float32
    BF16 = mybir.dt.bfloat16
    FP8 = mybir.dt.float8e4
    MULT = mybir.AluOpType.mult

    wt = nc.alloc_sbuf_tensor("wt", [128, 128], BF16).ap()
    em = nc.alloc_sbuf_tensor("em", [128, K * SW], FP8).ap()
    st0 = nc.alloc_sbuf_tensor("st0", [128, SW], BF16).ap()
    st = {g: [st0[:, GOFF[g]: GOFF[g] + GWID[g]],
              nc.alloc_sbuf_tensor(f"st{g}1", [128, GWID[g]], BF16).ap()]
          for g in range(NGRP)}
    ps = {g: [nc.alloc_psum_tensor(f"ps{g}{i}", [128, GWID[g]], FP32).ap()
              for i in range(2)] for g in range(NGRP)}

    with ExitStack() as ctx:
        tt_sem = [ctx.enter_context(nc.semaphore(f"tt{g}"))
                  for g in range(NGRP)]
        mm_sem = [ctx.enter_context(nc.semaphore(f"mm{g}"))
                  for g in range(NGRP)]
        qsy = ctx.enter_context(nc.semaphore("qsy"))
        qsc = ctx.enter_context(nc.semaphore("qsc"))
        # a-DMA completions go to `aq`, which nothing ever waits on, so a
        # completion racing the NEFF-epilogue sem zeroing is harmless
        aq = ctx.enter_context(nc.semaphore("aq"))

        # chunk j queue + completion count: even chunks on scalar
        # (chunk0=16, chunk2=32, chunk4=48), odd on sync behind wexp
        # (wexp=16, chunk1=32, chunk3=48, chunk5=64). Each queue's first
        # DMA is the one gate actually waited on early (HWDGE completions
        # are ~2us and serialize per ring).
        nchunk = len(_CHUNK_ROWS) - 1
        chunk_q = [(qsc, 16 * (j // 2 + 1)) if j % 2 == 0
                   else (qsy, 16 * (j // 2 + 2)) for j in range(nchunk)]
        chunk_of_row = {}
        for j in range(nchunk):
            chunk_of_row[_CHUNK_ROWS[j]] = j

        with nc.Block("crf", no_gpsimd_drain=True) as block:

            def f_sync(eng):
                eng.dma_start(wt, wexp).then_inc(qsy, 16)
                for j in range(1, nchunk, 2):
                    sl = slice(_CHUNK_ROWS[j] * SW, _CHUNK_ROWS[j + 1] * SW)
                    eng.dma_start(em[:, sl], em_slab[:, sl]).then_inc(qsy, 16)
                for g in (1, 3):
                    eng.wait_ge(tt_sem[g], NSTEP + 1)
                    eng.dma_start(out[:, GOFF[g]: GOFF[g] + GWID[g]],
                                  st[g][NSTEP % 2]).then_inc(aq, 16)

            def f_scalar(eng):
                for j in range(0, nchunk, 2):
                    sl = slice(_CHUNK_ROWS[j] * SW, _CHUNK_ROWS[j + 1] * SW)
                    eng.dma_start(em[:, sl], em_slab[:, sl]).then_inc(qsc, 16)
                for g in (0, 2):
                    eng.wait_ge(tt_sem[g], NSTEP + 1)
                    eng.dma_start(out[:, GOFF[g]: GOFF[g] + GWID[g]],
                                  st[g][NSTEP % 2]).then_inc(aq, 16)

            def f_tensor(eng):
                eng.wait_ge(qsy, 16)          # wexp loaded
                for k in range(NSTEP):
                    cur = k % 2
                    for g in range(NGRP):
                        mm = eng.matmul(ps[g][cur], wt, st[g][cur],
                                        start=True, stop=True)
                        # st[g][cur] written by tensor_scalar (k=0) / TT
                        _embed_wait(mybir, mm, tt_sem[g], k + 1)
                        mm.then_inc(mm_sem[g], 1)

            def f_vector(eng):
                # init: h_0 = slab row 0 (colsum(W') folded in on the host)
                eng.wait_ge(*chunk_q[0])
                for g in range(NGRP):
                    eng.tensor_copy(
                        st[g][0],
                        em[:, GOFF[g]: GOFF[g] + GWID[g]]).then_inc(
                            tt_sem[g], 1)
                for k in range(NSTEP):
                    cur = k % 2
                    r = k + 1                 # slab row consumed by step k
                    if r in chunk_of_row:
                        eng.wait_ge(*chunk_q[chunk_of_row[r]])
                    for g in range(NGRP):
                        emk = em[:, SW * r + GOFF[g]:
                                 SW * r + GOFF[g] + GWID[g]]
                        tt = eng.tensor_tensor(st[g][(k + 1) % 2], ps[g][cur],
                                               emk, MULT)
                        _embed_wait(mybir, tt, mm_sem[g], k + 1)
                        tt.then_inc(tt_sem[g], 1)

            # output-DMA completion and semaphore zeroing are covered by the
            # NEFF's end-of-program drains + full sem-file zeroing epilogue
            block.sync(f_sync)
            block.scalar(f_scalar)
            block.tensor(f_tensor)
            block.vector(f_vector)

    return nc


def _get_program():
    if "nc" not in _prog_cache:
        _prog_cache["nc"] = _build_program()
    return _prog_cache["nc"]


# ----------------------------------------------------------------------------
# host-side helpers
# ----------------------------------------------------------------------------

def _bf16_f32(x):
    import ml_dtypes
    return np.asarray(x, np.float32).astype(ml_dtypes.bfloat16).astype(np.float32)


def _dev5(arr):
    """arr[K, NCH, B, T] fp32 -> device layout [128, K*SW]:
    p = 64*(b//64) + j, free offset = SW*r + 64*ch + b%64."""
    a = arr.reshape(K, NCH, 2, 64, T)          # r, ch, bg, b64, j
    a = a.transpose(2, 4, 0, 1, 3)             # bg, j, r, ch, b64
    return np.ascontiguousarray(a).reshape(128, K * SW)


def _build_slabs(emissions, start_t, csum):
    """Per-core Em slabs exp(em) in device layout (slab row r of chain q =
    global em row 32q + r; cast to fp8 at upload). Row 0 is the chain's
    initial state: exp(em row) * colsum(W') = the true first-step image of
    the all-ones vector; chain 0's row 0 is instead an exp-space
    representative of alpha_0 = start + em[:, 0], shifted by mshift."""
    em32 = emissions.astype(np.float32)
    slabs = np.empty((NCORE, K, NCH, B, T), np.float32)
    for c in range(NCORE):
        for ch in range(NCH):
            t0 = K * (NCH * c + ch)
            slabs[c, :, ch] = np.exp(em32[:, t0: t0 + K].transpose(1, 0, 2))
            slabs[c, 0, ch] *= csum[None, :]
    a0 = start_t[None, :].astype(np.float32) + em32[:, 0]
    mshift = np.float32(a0.max() - 5.3)
    slabs[0, 0, 0] = np.exp(a0 - mshift)
    # device decodes fp8e4 as IEEE e4m3 (inf at >=256); clamp to stay finite
    np.minimum(slabs, np.float32(224.0), out=slabs)
    return np.stack([_dev5(slabs[c]) for c in range(NCORE)]), mshift


def _lse64(v):
    m = v.max(-1)
    return m + np.log(np.exp(v - m[..., None]).sum(-1))


def _host_score(emissions, tags, transitions, start_t, end_t, mask):
    em64 = emissions.astype(np.float64)
    W64 = transitions.astype(np.float64)
    maskf = mask.astype(np.float64)
    emit = np.take_along_axis(em64, tags[..., None].astype(np.int64),
                              axis=2)[..., 0]
    trans = W64[tags[:, 1:], tags[:, :-1]]
    score = (start_t.astype(np.float64)[tags[:, 0]] + emit[:, 0]
             + ((trans + emit[:, 1:]) * maskf[:, 1:]).sum(1))
    last_idx = maskf.sum(1).astype(np.int64) - 1
    last_tags = np.take_along_axis(tags, last_idx[:, None], axis=1)[:, 0]
    return score + end_t.astype(np.float64)[last_tags]


def _fallback_reference(emissions, tags, mask, transitions, start_t, end_t):
    """Exact host computation (only used if mask is not all ones)."""
    em = emissions.astype(np.float64)
    Wt = transitions.astype(np.float64)
    alpha = start_t.astype(np.float64)[None, :] + em[:, 0]
    for t in range(1, S):
        x = alpha[:, :, None] + Wt[None]
        m = x.max(1)
        na = m + np.log(np.exp(x - m[:, None, :]).sum(1)) + em[:, t]
        alpha = np.where(mask[:, t][:, None], na, alpha)
    logZ = _lse64(alpha + end_t.astype(np.float64)[None, :])
    score = _host_score(emissions, tags, transitions, start_t, end_t, mask)
    return np.float32(-(score - logZ).mean())


# ----------------------------------------------------------------------------
# entry point
# ----------------------------------------------------------------------------

def kernel(emissions, tags, mask, transitions, start_transitions,
           end_transitions):
    global _last_results
    emissions = np.asarray(emissions, np.float32)
    tags = np.asarray(tags)
    mask = np.asarray(mask)
    transitions = np.asarray(transitions, np.float32)
    start_t = np.asarray(start_transitions, np.float32)
    end_t = np.asarray(end_transitions, np.float32)

    if not mask.all():
        return _fallback_reference(emissions, tags, mask, transitions,
                                   start_t, end_t)

    # --- host prep ---
    import ml_dtypes
    Wexp2 = np.zeros((128, 128), np.float32)
    Wexp2[:64, :64] = np.exp(transitions - C0)
    Wexp2[64:, 64:] = Wexp2[:64, :64]
    Wd = _bf16_f32(Wexp2)
    csum_dev = Wd[:64, :64].sum(0).astype(np.float32)
    slabs, mshift = _build_slabs(emissions, start_t, csum_dev)

    in_maps = [{"em_slab": slabs[c].astype(ml_dtypes.float8_e4m3fn),
                "wexp": Wd.astype(ml_dtypes.bfloat16)}
               for c in range(NCORE)]

    # --- device run ---
    import os
    from concourse.bass_utils import run_bass_kernel_spmd
    nc = _get_program()
    res = run_bass_kernel_spmd(
        nc, in_maps, list(range(NCORE)),
        trace=bool(os.environ.get("CRF_TRACE")),
    )
    _last_results = res

    # --- unpack: chain q = NCH*core + ch at out cols [64*ch, 64*ch+64) ---
    a = np.zeros((NCHAIN, B, T), np.float64)
    for core in range(NCORE):
        o = np.asarray(res.results[core]["out"], np.float32)
        for ch in range(NCH):
            q = NCH * core + ch
            at_ = o[:, 64 * ch: 64 * ch + 64]
            for bg in range(2):
                a[q, 64 * bg: 64 * bg + 64] = at_[64 * bg: 64 * bg + 64].T

    # --- stitch (fp64); chain q's init is the first-step image of the
    #     all-ones vector, whose lse is log T ---
    with np.errstate(divide="ignore"):
        la = np.log(a)
    gam = np.zeros(B)
    La = la[0] + float(C0) * NSTEP + float(mshift)
    for q in range(1, NCHAIN):
        gam = gam + _lse64(La) - np.log(T)
        La = la[q] + float(C0) * K
    logZ = _lse64(La + end_t.astype(np.float64)[None, :]) + gam

    score = _host_score(emissions, tags, transitions, start_t, end_t, mask)
    return np.float32(-(score - logZ).mean())
